# revision 17
# baseline (speedup 1.0000x reference)
"""CompressionTransformerLayer on 8 TRN2 NeuronCores (Bass/Tile) — v2.

2-D sharding: core c -> batch beta = c//2; within the batch pair mu = c%2
selects BOTH the head-group (heads mu*8..mu*8+7) for self/cross attention
and the token-half (tokens mu*128..mu*128+127 of the batch's 256) for the
FFN tail and the final output slice.

Per core:
 - self-attn: LN1 + in-proj for its 8 heads over the batch's 256 queries,
   softmax via appended-ones denominator, pair AllGather of head outputs,
   replicated (pair-identical) out-proj + residual -> x kept in SBUF.
 - cross-attn: streams the batch's 8192-token context in 512-token chunks,
   projects K/V for its 8 heads (N=512 matmuls), transposed-scores softmax
   with PSUM-resident accumulators for all 8 heads across the whole
   context, pair AllGather of head outputs.
 - tail: replicated co-proj + residual -> x3 token tiles, one-hot fp32
   matmul selects the core's own 128 tokens, LN3 + FFN on them.
All transposes are PE transposes (identity matmul); the residual stream
never leaves SBUF.  Matmuls bf16 with fp32 PSUM; LN/softmax/residual fp32.
"""
import sys
sys.path.insert(0, "/opt/trn_rl_repo")
sys.path.insert(0, "/root/.axon_site")

import contextlib
import os
import numpy as np

import concourse.bass as bass
import concourse.mybir as mybir
import concourse.tile as tile
from concourse import bacc
from concourse.bass_utils import run_bass_kernel_spmd
from concourse.masks import make_identity

f32, bf16 = mybir.dt.float32, mybir.dt.bfloat16
AF = mybir.ActivationFunctionType
ALU = mybir.AluOpType
BF16NP = mybir.dt.np(bf16)

D, H, HD, DFF = 1024, 16, 64, 4096
B, Q, S = 4, 256, 8192
NC = 8
HPC = 8              # heads per core
QF = HPC * HD        # 512 qkv features per core
NG = QF // 128       # 4 feature groups per core
FO = D // 128        # 8 feature tiles of d_model
SCH = 512            # context chunk (tokens)
NSC = S // SCH       # 16 chunks
KPF = 2              # cross K/V pipeline prefill depth
EPS = 1e-5

_CACHE = {}


def _build():
    nc = bacc.Bacc("TRN2", target_bir_lowering=False, debug=False,
                   enable_asserts=True, num_devices=NC)

    def din(name, shape, dt=bf16):
        return nc.dram_tensor(name, shape, dt, kind="ExternalInput").ap()

    queries = din("queries", [128, 2, D], f32)       # batch's 256 tokens
    ctx = din("ctx", [D, S], bf16)                   # batch's context, feature-major
    ln1 = din("ln1", [128, FO, 2], f32)              # [:, :, 0]=g, [:, :, 1]=b
    ln2 = din("ln2", [128, FO, 2], f32)
    ln3 = din("ln3", [128, FO, 2], f32)
    wq = din("wq", [D, QF]); wk = din("wk", [D, QF]); wv = din("wv", [D, QF])
    bqkv = din("bqkv", [128, 3, NG], f32)            # packed q/k/v biases
    saow = din("saow", [D, D]); saob = din("saob", [D], f32)
    cqw = din("cqw", [D, QF]); ckw = din("ckw", [D, QF]); cvw = din("cvw", [D, QF])
    bckv = din("bckv", [128, 3, NG], f32)            # packed cq/ck/cv biases
    cow = din("cow", [D, D]); cob = din("cob", [D], f32)
    w1 = din("w1", [D, DFF]); b1 = din("b1", [DFF], f32)
    w2 = din("w2", [DFF, D]); b2 = din("b2", [D], f32)
    # own-token one-hot: psel[r, tt, p] = 1 iff tt == mu and r == p
    psel = din("psel", [128, 2, 128], f32)

    out = nc.dram_tensor("out", [128, D], f32, kind="ExternalOutput").ap()
    DBG = bool(os.environ.get("BASSDBG"))
    dbg = {}
    if DBG:
        for nm, shp, dt in [("dbg_qnT", [128, FO, 256], bf16),
                            ("dbg_oself", [128, NG, 256], bf16),
                            ("dbg_x", [128, 2, D], f32),
                            ("dbg_q2T", [128, NG, 256], bf16),
                            ("dbg_ocross", [128, NG, 256], bf16),
                            ("dbg_x3own", [128, D], f32),
                            ("dbg_h", [128, DFF], bf16)]:
            dbg[nm] = nc.dram_tensor(nm, shp, dt, kind="ExternalOutput").ap()

    ctx_r = ctx.rearrange("(fo fi) t -> fi fo t", fi=128)
    saow_r = saow.rearrange("(fo fi) n -> fi fo n", fi=128)
    cow_r = cow.rearrange("(fo fi) n -> fi fo n", fi=128)
    w1_r = w1.rearrange("(fo fi) n -> fi fo n", fi=128)
    w2_r = w2.rearrange("(dg fi) n -> fi dg n", fi=128)
    wsl_r = [w.rearrange("(fo fi) o -> fi fo o", fi=128)
             for w in (wq, wk, wv, cqw, ckw, cvw)]

    with tile.TileContext(nc) as tc:
        with contextlib.ExitStack() as ctxs:
            const = ctxs.enter_context(tc.tile_pool(name="const", bufs=1))
            fm = ctxs.enter_context(tc.tile_pool(name="fm", bufs=1))
            sb = ctxs.enter_context(tc.tile_pool(name="sb", bufs=3))
            pTp = ctxs.enter_context(tc.tile_pool(name="pTp", bufs=6))
            cstream = ctxs.enter_context(tc.tile_pool(name="cstream", bufs=2))
            kvs = ctxs.enter_context(tc.tile_pool(name="kvs", bufs=3))
            wstream = ctxs.enter_context(tc.tile_pool(name="wstream", bufs=2))
            dram = ctxs.enter_context(tc.tile_pool(name="dram", bufs=1, space="DRAM"))
            psKV = ctxs.enter_context(tc.tile_pool(name="psKV", bufs=2, space="PSUM"))
            psS = ctxs.enter_context(tc.tile_pool(name="psS", bufs=2, space="PSUM"))
            psO = ctxs.enter_context(tc.tile_pool(name="psO", bufs=4, space="PSUM"))

            def ldconst(ap_, shape, dt, name):
                t = const.tile(shape, dt, tag=name, name=name)
                nc.sync.dma_start(t[:], ap_)
                return t

            ident = const.tile([128, 128], bf16, name="ident")
            make_identity(nc, ident[:])

            ckw_sb = ldconst(wsl_r[4], [128, FO, QF], bf16, "ckw_sb")
            cvw_sb = ldconst(wsl_r[5], [128, FO, QF], bf16, "cvw_sb")
            ln1_sb = ldconst(ln1, [128, FO, 2], f32, "ln1_sb")
            ln2_sb = ldconst(ln2, [128, FO, 2], f32, "ln2_sb")
            ln3_sb = ldconst(ln3, [128, FO, 2], f32, "ln3_sb")
            bqkv_sb = ldconst(bqkv, [128, 3, NG], f32, "bqkv_sb")
            bckv_sb = ldconst(bckv, [128, 3, NG], f32, "bckv_sb")
            psel_sb = ldconst(psel, [128, 2, 128], f32, "psel_sb")

            def bcast_vec(ap_, n, name):
                full = const.tile([128, n], f32, tag=name, name=name)
                nc.sync.dma_start(full[:], ap_[None, :].to_broadcast((128, n)))
                return full

            saob_bc = bcast_vec(saob, D, "saob_bc")
            cob_bc = bcast_vec(cob, D, "cob_bc")
            b2_bc = bcast_vec(b2, D, "b2_bc")
            b1_bc = bcast_vec(b1, DFF, "b1_bc")

            q_sb = fm.tile([128, 2, D], f32, name="q_sb")
            nc.sync.dma_start(q_sb[:], queries)

            # ---- helpers ----
            def ln_norm(dst, src, ntt, nm):
                """Per-token normalize (no gamma/beta): src [128, ntt, 1024] f32
                -> dst [128, ntt, 1024] bf16."""
                for tt in range(ntt):
                    stats = sb.tile([128, 2, 6], f32, tag="ln_st", name=f"lns_{nm}{tt}")
                    nc.vector.bn_stats(stats[:, 0, :], src[:, tt, 0:512])
                    nc.vector.bn_stats(stats[:, 1, :], src[:, tt, 512:1024])
                    mv = sb.tile([128, 2], f32, tag="ln_mv", name=f"lnm_{nm}{tt}")
                    nc.vector.bn_aggr(mv[:], stats[:])
                    eps = sb.tile([128, 1], f32, tag="ln_eps", name=f"lne_{nm}{tt}")
                    nc.vector.memset(eps[:], EPS)
                    rstd = sb.tile([128, 1], f32, tag="ln_rs", name=f"lnr_{nm}{tt}")
                    nc.scalar.activation(rstd[:], mv[:, 1:2], AF.Sqrt, bias=eps[:], scale=1.0)
                    nc.vector.reciprocal(rstd[:], rstd[:])
                    nc.vector.tensor_scalar(dst[:, tt, :], src[:, tt, :],
                                            scalar1=mv[:, 0:1], scalar2=rstd[:],
                                            op0=ALU.subtract, op1=ALU.mult)

            def pe_transpose(dst, src, ntt, g_sb, nm):
                """src [128, ntt, 1024] bf16 token-major -> dst [128, FO, ntt*128]
                feature-major, applying gamma/beta (g_sb [128, FO, 2]) if given."""
                for tt in range(ntt):
                    for fo in range(FO):
                        pt = psS.tile([128, 128], bf16, tag="pss", name=f"pt_{nm}{tt}_{fo}")
                        nc.tensor.transpose(pt[:], src[:, tt, fo * 128:(fo + 1) * 128], ident[:])
                        dsl = dst[:, fo, tt * 128:(tt + 1) * 128]
                        if g_sb is not None:
                            nc.vector.tensor_scalar(dsl, pt[:],
                                                    scalar1=g_sb[:, fo, 0:1],
                                                    scalar2=g_sb[:, fo, 1:2],
                                                    op0=ALU.mult, op1=ALU.add)
                        else:
                            nc.vector.tensor_copy(dsl, pt[:])

            # ---- P1: LN1(queries) -> qn -> qn_T ----
            qn = fm.tile([128, 2, D], bf16, name="qn")
            ln_norm(qn, q_sb, 2, "l1")
            qn_T = fm.tile([128, FO, 256], bf16, name="qn_T")
            pe_transpose(qn_T, qn, 2, ln1_sb, "qn")
            if DBG:
                nc.sync.dma_start(dbg["dbg_qnT"][:], qn_T[:])

            # ---- P2: self qkv (8 heads) ----
            q_T = fm.tile([128, NG, 256], bf16, name="q_T")
            k_T = fm.tile([128, NG, 256], bf16, name="k_T")
            for wi, dst in ((0, q_T), (1, k_T)):
                wsb = wstream.tile([128, FO, QF], bf16, tag="wc", name=f"w_qk{wi}")
                nc.sync.dma_start(wsb[:], wsl_r[wi])
                for g in range(NG):
                    ps = psKV.tile([128, 256], f32, tag="ps", name=f"ps_qk{wi}_{g}")
                    for f in range(FO):
                        nc.tensor.matmul(ps[:], wsb[:, f, g * 128:(g + 1) * 128],
                                         qn_T[:, f, :], start=(f == 0), stop=(f == FO - 1))
                    nc.scalar.activation(dst[:, g, :], ps[:], AF.Identity,
                                         bias=bqkv_sb[:, wi, g:g + 1])
            # v token-major -> vaug_self [128, kt, h, 65]
            vaug_s = fm.tile([128, 2, HPC, 65], bf16, name="vaug_s")
            nc.vector.memset(vaug_s[:, :, :, 64:65], 1.0)
            wv_sb = wstream.tile([128, FO, QF], bf16, tag="wc", name="w_v")
            nc.sync.dma_start(wv_sb[:], wsl_r[2])
            for kt in range(2):
                ps = psKV.tile([128, QF], f32, tag="ps", name=f"ps_vs{kt}")
                for f in range(FO):
                    nc.tensor.matmul(ps[:], qn_T[:, f, kt * 128:(kt + 1) * 128],
                                     wv_sb[:, f, :], start=(f == 0), stop=(f == FO - 1))
                nc.vector.tensor_copy(
                    vaug_s[:, kt, :, 0:64],
                    ps[:].rearrange("p (h d) -> p h d", h=HPC))

            # ---- P3: self-attention (scores/exp/attnV), pipelined heads ----
            o_self = fm.tile([128, NG, 256], bf16, name="o_self")

            def head_rows(h):
                r0 = (h % 2) * 64
                return slice(r0, r0 + 64), h // 2

            def self_score(h):
                rows, g = head_rows(h)
                pts = []
                pss = psS.tile([128, 2, 256], f32, tag="pss", name=f"pssS_{h}")
                for kt in range(2):
                    nc.tensor.matmul(pss[:, kt, :], k_T[rows, g, kt * 128:(kt + 1) * 128],
                                     q_T[rows, g, :], start=True, stop=True)
                    pT = pTp.tile([128, 256], bf16, tag="pT", name=f"pTS_{h}_{kt}")
                    nc.scalar.activation(pT[:], pss[:, kt, :], AF.Exp, scale=0.125)
                    pts.append(pT)
                return pts

            def self_attnv(h, pts):
                pso = psO.tile([65, 256], f32, tag="pso", name=f"psoS_{h}")
                for kt in range(2):
                    nc.tensor.matmul(pso[:], vaug_s[:, kt, h, 0:65], pts[kt][:],
                                     start=(kt == 0), stop=(kt == 1))
                return pso

            def finalize(h, pso, pso_sl, o_dst, bias_sb, bias_col, nm):
                rows, g = head_rows(h)
                rinv = sb.tile([1, 256], f32, tag="rinv", name=f"ri{nm}_{h}")
                nc.vector.reciprocal(rinv[:], pso[64:65, pso_sl, :] if pso_sl is not None
                                     else pso[64:65, :])
                rb = sb.tile([64, 256], f32, tag="rb", name=f"rb{nm}_{h}")
                nc.gpsimd.partition_broadcast(rb[:], rinv[:])
                osl = o_dst[rows, g, :]
                num = pso[0:64, pso_sl, :] if pso_sl is not None else pso[0:64, :]
                nc.vector.tensor_tensor(osl, num, rb[:], ALU.mult)
                nc.vector.tensor_scalar_add(osl, osl, bias_sb[rows, bias_col, g:g + 1])

            from collections import deque as _deque
            spend = _deque()
            for h in range(HPC):
                spend.append((h, self_score(h)))
                while len(spend) > 2:
                    hh, pts = spend.popleft()
                    finalize(hh, self_attnv(hh, pts), None, o_self, bqkv_sb, 2, "S")
            while spend:
                hh, pts = spend.popleft()
                finalize(hh, self_attnv(hh, pts), None, o_self, bqkv_sb, 2, "S")
            if DBG:
                nc.sync.dma_start(dbg["dbg_oself"][:], o_self[:])

            # ---- AG#1 (pair) ----
            PAIRS = [[0, 1], [2, 3], [4, 5], [6, 7]]
            ag1_in = dram.tile([128, NG, 256], bf16)
            ag1_out = dram.tile([2, 128, NG, 256], bf16)
            nc.sync.dma_start(ag1_in[:], o_self[:])
            nc.gpsimd.collective_compute(
                "AllGather", ALU.bypass, replica_groups=PAIRS,
                ins=[ag1_in[:].opt()], outs=[ag1_out[:].opt()])

            # ---- cross K/V chunk pipeline (emitted early to cover AG) ----
            kc_tiles = {}
            vaug_tiles = {}

            def emit_kv(sc):
                ctx_T = cstream.tile([128, FO, SCH], bf16, tag="ctxT", name=f"ctxT{sc}")
                nc.sync.dma_start(ctx_T[:], ctx_r[:, :, sc * SCH:(sc + 1) * SCH])
                kc = kvs.tile([128, NG, SCH], bf16, tag="kc", name=f"kc{sc}")
                for g in range(NG):
                    ps = psKV.tile([128, SCH], f32, tag="ps", name=f"ps_k{sc}_{g}")
                    for f in range(FO):
                        nc.tensor.matmul(ps[:], ckw_sb[:, f, g * 128:(g + 1) * 128],
                                         ctx_T[:, f, :], start=(f == 0), stop=(f == FO - 1))
                    nc.vector.tensor_scalar_add(kc[:, g, :], ps[:],
                                                bckv_sb[:, 1, g:g + 1])
                vaug = kvs.tile([128, 4, HPC, 65], bf16, tag="vaug", name=f"vaug{sc}")
                nc.vector.memset(vaug[:, :, :, 64:65], 1.0)
                for kt in range(4):
                    ps = psKV.tile([128, QF], f32, tag="ps", name=f"ps_v{sc}_{kt}")
                    for f in range(FO):
                        nc.tensor.matmul(ps[:], ctx_T[:, f, kt * 128:(kt + 1) * 128],
                                         cvw_sb[:, f, :], start=(f == 0), stop=(f == FO - 1))
                    nc.vector.tensor_copy(
                        vaug[:, kt, :, 0:64],
                        ps[:].rearrange("p (h d) -> p h d", h=HPC))
                kc_tiles[sc] = kc
                vaug_tiles[sc] = vaug

            for sc in range(KPF):
                emit_kv(sc)

            # ---- P4: sa_out proj (full 256 tokens) + residual -> x ----
            o_full_T = fm.tile([128, FO, 256], bf16, name="o_full_T")
            for r in range(2):
                nc.sync.dma_start(o_full_T[:, r * NG:(r + 1) * NG, :], ag1_out[r])
            x = fm.tile([128, 2, D], f32, name="x")
            for oc in range(2):
                saow_c = wstream.tile([128, FO, 512], bf16, tag="wc", name=f"saow{oc}")
                nc.sync.dma_start(saow_c[:], saow_r[:, :, oc * 512:(oc + 1) * 512])
                for tt in range(2):
                    ps = psKV.tile([128, 512], f32, tag="ps", name=f"ps_x{oc}_{tt}")
                    for s in range(FO):
                        nc.tensor.matmul(ps[:], o_full_T[:, s, tt * 128:(tt + 1) * 128],
                                         saow_c[:, s, :], start=(s == 0), stop=(s == FO - 1))
                    xs = x[:, tt, oc * 512:(oc + 1) * 512]
                    nc.vector.tensor_tensor(xs, ps[:], q_sb[:, tt, oc * 512:(oc + 1) * 512],
                                            ALU.add)
                    nc.vector.tensor_tensor(xs, xs, saob_bc[:, oc * 512:(oc + 1) * 512],
                                            ALU.add)
            if DBG:
                nc.sync.dma_start(dbg["dbg_x"][:], x[:])

            # ---- P5: LN2 -> xn2_T -> q2 (own heads) ----
            xn2 = fm.tile([128, 2, D], bf16, name="xn2")
            ln_norm(xn2, x, 2, "l2")
            xn2_T = fm.tile([128, FO, 256], bf16, name="xn2_T")
            pe_transpose(xn2_T, xn2, 2, ln2_sb, "x2")
            q2_T = fm.tile([128, NG, 256], bf16, name="q2_T")
            cqw_sb = wstream.tile([128, FO, QF], bf16, tag="wc", name="w_cq")
            nc.sync.dma_start(cqw_sb[:], wsl_r[3])
            for g in range(NG):
                ps = psKV.tile([128, 256], f32, tag="ps", name=f"ps_q2{g}")
                for f in range(FO):
                    nc.tensor.matmul(ps[:], cqw_sb[:, f, g * 128:(g + 1) * 128],
                                     xn2_T[:, f, :], start=(f == 0), stop=(f == FO - 1))
                nc.scalar.activation(q2_T[:, g, :], ps[:], AF.Identity,
                                     bias=bckv_sb[:, 0, g:g + 1])
            if DBG:
                nc.sync.dma_start(dbg["dbg_q2T"][:], q2_T[:])

            # ---- P6: cross-attention over all chunks ----
            # 8 heads packed 2-per-PSUM-bank: pso_g[g] is [65, 2, 256]
            pso_g = [psO.tile([65, 2, 256], f32, tag="pso", name=f"psoC{g}")
                     for g in range(NG)]

            def cross_attnv(sc, h, kt, pT):
                g = h // 2
                nc.tensor.matmul(pso_g[g][:, h % 2, :], vaug_tiles[sc][:, kt, h, 0:65],
                                 pT[:], start=(sc == 0 and kt == 0),
                                 stop=(sc == NSC - 1 and kt == 3))

            from collections import deque
            pending = deque()
            for sc in range(NSC):
                if sc + KPF < NSC:
                    emit_kv(sc + KPF)
                for kt in range(4):
                    for ph in range(HPC // 2):
                        pss = psS.tile([128, 2, 256], f32, tag="pss",
                                       name=f"pssC{sc}_{ph}_{kt}")
                        for j in range(2):
                            h = 2 * ph + j
                            rows, g = head_rows(h)
                            nc.tensor.matmul(
                                pss[:, j, :], kc_tiles[sc][rows, g, kt * 128:(kt + 1) * 128],
                                q2_T[rows, g, :], start=True, stop=True)
                            pT = pTp.tile([128, 256], bf16, tag="pT",
                                          name=f"pTC{sc}_{h}_{kt}")
                            nc.scalar.activation(pT[:], pss[:, j, :], AF.Exp, scale=0.125)
                            pending.append((sc, h, kt, pT))
                        while len(pending) > 2:
                            cross_attnv(*pending.popleft())
                if sc >= KPF:
                    kc_tiles.pop(sc - KPF); vaug_tiles.pop(sc - KPF)
            while pending:
                cross_attnv(*pending.popleft())

            o_cross = fm.tile([128, NG, 256], bf16, name="o_cross")
            for h in range(HPC):
                finalize(h, pso_g[h // 2], h % 2, o_cross, bckv_sb, 2, "C")
            if DBG:
                nc.sync.dma_start(dbg["dbg_ocross"][:], o_cross[:])

            # ---- AG#2 (pair) ----
            ag2_in = dram.tile([128, NG, 256], bf16)
            ag2_out = dram.tile([2, 128, NG, 256], bf16)
            nc.sync.dma_start(ag2_in[:], o_cross[:])
            nc.gpsimd.collective_compute(
                "AllGather", ALU.bypass, replica_groups=PAIRS,
                ins=[ag2_in[:].opt()], outs=[ag2_out[:].opt()])
            oc_full_T = fm.tile([128, FO, 256], bf16, name="oc_full_T")
            for r in range(2):
                nc.sync.dma_start(oc_full_T[:, r * NG:(r + 1) * NG, :], ag2_out[r])

            # ---- P7: co proj + residual -> x3 tiles; select own tokens ----
            x3_own = fm.tile([128, 1, D], f32, name="x3_own")
            for oc in range(2):
                cow_c = wstream.tile([128, FO, 512], bf16, tag="wc", name=f"cow{oc}")
                nc.sync.dma_start(cow_c[:], cow_r[:, :, oc * 512:(oc + 1) * 512])
                x3t = []
                for tt in range(2):
                    ps = psKV.tile([128, 512], f32, tag="ps", name=f"ps_x3{oc}_{tt}")
                    for s in range(FO):
                        nc.tensor.matmul(ps[:], oc_full_T[:, s, tt * 128:(tt + 1) * 128],
                                         cow_c[:, s, :], start=(s == 0), stop=(s == FO - 1))
                    xt = sb.tile([128, 512], f32, tag="x3t", name=f"x3t{oc}_{tt}")
                    nc.vector.tensor_tensor(xt[:], ps[:], x[:, tt, oc * 512:(oc + 1) * 512],
                                            ALU.add)
                    nc.vector.tensor_tensor(xt[:], xt[:], cob_bc[:, oc * 512:(oc + 1) * 512],
                                            ALU.add)
                    x3t.append(xt)
                ps = psKV.tile([128, 512], f32, tag="ps", name=f"ps_sel{oc}")
                for tt in range(2):
                    nc.tensor.matmul(ps[:], psel_sb[:, tt, :], x3t[tt][:],
                                     start=(tt == 0), stop=(tt == 1))
                nc.vector.tensor_copy(x3_own[:, 0, oc * 512:(oc + 1) * 512], ps[:])
            if DBG:
                nc.sync.dma_start(dbg["dbg_x3own"][:], x3_own[:, 0, :])

            # ---- P8: LN3 + FFN on own 128 tokens ----
            xn3 = fm.tile([128, 1, D], bf16, name="xn3")
            ln_norm(xn3, x3_own, 1, "l3")
            xn3_T = fm.tile([128, FO, 128], bf16, name="xn3_T")
            pe_transpose(xn3_T, xn3, 1, ln3_sb, "x3")

            h_sb = fm.tile([128, DFF], bf16, name="h_sb")
            for dc in range(8):
                w1c = wstream.tile([128, FO, 512], bf16, tag="wc", name=f"w1c{dc}")
                nc.sync.dma_start(w1c[:], w1_r[:, :, dc * 512:(dc + 1) * 512])
                psh = psKV.tile([128, 512], f32, tag="ps", name=f"ps_h{dc}")
                for f in range(FO):
                    nc.tensor.matmul(psh[:], xn3_T[:, f, :], w1c[:, f, :],
                                     start=(f == 0), stop=(f == FO - 1))
                nc.vector.tensor_tensor(psh[:], psh[:],
                                        b1_bc[:, dc * 512:(dc + 1) * 512], ALU.add)
                nc.scalar.activation(h_sb[:, dc * 512:(dc + 1) * 512], psh[:], AF.Gelu)
            if DBG:
                nc.sync.dma_start(dbg["dbg_h"][:], h_sb[:])

            h_T = fm.tile([128, 32, 128], bf16, name="h_T")
            for dt in range(32):
                pt = psS.tile([128, 128], bf16, tag="pss", name=f"pt_h{dt}")
                nc.tensor.transpose(pt[:], h_sb[:, dt * 128:(dt + 1) * 128], ident[:])
                nc.vector.tensor_copy(h_T[:, dt, :], pt[:])

            out_sb = fm.tile([128, D], f32, name="out_sb")
            for oc in range(2):
                psy = psKV.tile([128, 512], f32, tag="ps", name=f"ps_y{oc}")
                for wc in range(4):
                    w2c = wstream.tile([128, FO, 512], bf16, tag="wc", name=f"w2c{oc}_{wc}")
                    nc.sync.dma_start(w2c[:], w2_r[:, wc * FO:(wc + 1) * FO,
                                                   oc * 512:(oc + 1) * 512])
                    for j in range(FO):
                        dt = wc * FO + j
                        nc.tensor.matmul(psy[:], h_T[:, dt, :], w2c[:, j, :],
                                         start=(dt == 0), stop=(dt == 31))
                ys = out_sb[:, oc * 512:(oc + 1) * 512]
                nc.vector.tensor_tensor(ys, psy[:], x3_own[:, 0, oc * 512:(oc + 1) * 512],
                                        ALU.add)
                nc.vector.tensor_tensor(ys, ys, b2_bc[:, oc * 512:(oc + 1) * 512], ALU.add)
            nc.sync.dma_start(out[:], out_sb[:])

    nc.compile()
    return nc


def _get_nc():
    if "nc" not in _CACHE:
        _CACHE["nc"] = _build()
    return _CACHE["nc"]


def kernel(**inputs):
    nc = _get_nc()
    inp = {k: np.asarray(v) for k, v in inputs.items()}

    def bf(a):
        return np.ascontiguousarray(a).astype(BF16NP)

    def pack_ln(g, b):
        # [D] -> [128, FO, 2]
        g = np.asarray(g, np.float32).reshape(FO, 128).T
        b = np.asarray(b, np.float32).reshape(FO, 128).T
        return np.ascontiguousarray(np.stack([g, b], axis=-1))

    def pack_b(v):
        # [QF] -> [128, NG]
        return np.asarray(v, np.float32).reshape(NG, 128).T

    sa_in_w = inp["sa_in_w"]; sa_in_b = inp["sa_in_b"]
    shared = {
        "ln1": pack_ln(inp["ln1_g"], inp["ln1_b"]),
        "ln2": pack_ln(inp["ln2_g"], inp["ln2_b"]),
        "ln3": pack_ln(inp["ln3_g"], inp["ln3_b"]),
        "saow": bf(inp["sa_out_w"].T), "saob": np.asarray(inp["sa_out_b"], np.float32),
        "cow": bf(inp["co_w"].T), "cob": np.asarray(inp["co_b"], np.float32),
        "w1": bf(inp["w1"].T), "b1": np.asarray(inp["b1"], np.float32),
        "w2": bf(inp["w2"].T), "b2": np.asarray(inp["b2"], np.float32),
    }
    in_maps = []
    eye = np.eye(128, dtype=np.float32)
    for c in range(NC):
        beta, mu = c // 2, c % 2
        r = slice(mu * QF, (mu + 1) * QF)
        psel = np.zeros((128, 2, 128), np.float32)
        psel[:, mu, :] = eye
        m = dict(shared)
        m.update({
            "queries": np.ascontiguousarray(
                inp["queries"][beta].reshape(2, 128, D).swapaxes(0, 1).astype(np.float32)),
            "ctx": bf(inp["context"][beta].T),
            "wq": bf(sa_in_w[0 * D:1 * D][r].T),
            "wk": bf(sa_in_w[1 * D:2 * D][r].T),
            "wv": bf(sa_in_w[2 * D:3 * D][r].T),
            "bqkv": np.ascontiguousarray(np.stack(
                [pack_b(sa_in_b[i * D:i * D + D][r]) for i in range(3)], axis=1)),
            "cqw": bf(inp["cq_w"][r].T), "ckw": bf(inp["ck_w"][r].T),
            "cvw": bf(inp["cv_w"][r].T),
            "bckv": np.ascontiguousarray(np.stack(
                [pack_b(inp[nm][r]) for nm in ("cq_b", "ck_b", "cv_b")], axis=1)),
            "psel": psel,
        })
        in_maps.append(m)

    res = run_bass_kernel_spmd(nc, in_maps, core_ids=list(range(NC)),
                               **_CACHE.get("run_kwargs", {}))
    _CACHE["last_result"] = res
    out = np.concatenate([np.asarray(res.results[c]["out"]) for c in range(NC)], axis=0)
    return out.reshape(B, Q, D).astype(np.float32)
